# revision 42
# baseline (speedup 1.0000x reference)
"""MHSA + RoPE kernel for Trainium2, 8 NeuronCores.

Sharding: data-parallel over batch (B=2) x tensor-parallel over heads
(16 heads -> 4 head-groups of 4). Core c handles batch c//4, heads
[4*(c%4) : 4*(c%4)+4]. Each core computes its partial o_proj output
[N, D]; host sums the 4 partials per batch (the "all-reduce").

Design (arrived at via NTFF traces; 778us -> 393us -> this):
  - Everything on the PE runs bf16 (f32r is ~2x slower per row and
    power-throttles harder).
  - ONE kernel-lifetime PSUM pool with four 2-bank tags (s0,s1,a0,a1).
    Tile pool *transitions* gate on the prior pool's full release, so a
    single pool + manual tag rings removes all PSUM phase barriers.
  - HAM pre-warm: dummy matmuls + exp-table preload fill the startup
    DMA wait so the PE clock gate is 8/8 and the ACT table is resident
    before real work.
  - qk projections: 8 interleaved PSUM chains (two per tag tile);
    RoPE is fused per (head, n-block) right behind each PSUM drain
    (subtile deps make this legal), with the d-half swap done by
    SBUF-SBUF DMAs on the sync queue. RoPE is fully done ~2us after
    the last qk matmul instead of pacing the whole v phase.
  - v projection: 4 chains on the a-tags; drains on DVE. The first
    PRE=16 attention score+exp steps run interleaved inside the v
    t-loop (ActE is otherwise idle there), buffered in [128,2048]
    s_exp pair tiles, which shortens the exp-paced attention phase.
  - attention: one software-pipelined stream; scores run PRE steps
    ahead of the consume stream. exp on ActE is the pacer (~1.0us per
    [128,1024]). Softmax denominators accumulate in bf16 on DVE (one
    [128,2048] add per step-pair), are reduced by a ones-column
    matmul, inverted (fast reciprocal), partition-broadcast on GpSimd
    and applied on DVE - with the reduce/apply DEFERRED 2/6 steps into
    the next window so the in-order PE/DVE queues never head-block on
    the accumulator chain at a window boundary.
  - o_proj in bf16 with per-chunk output DMA on the last tile.
Output is bf16; the host sums the 4 TP partials per batch in f32.
"""

import sys

sys.path.insert(0, "/opt/trn_rl_repo")

import numpy as np
import ml_dtypes

import concourse.bass as bass
import concourse.tile as tile
from concourse import bacc, mybir
from concourse.bass_utils import run_bass_kernel_spmd

F32 = mybir.dt.float32
BF16 = mybir.dt.bfloat16
MULT = mybir.AluOpType.mult
ADD = mybir.AluOpType.add
EXP = mybir.ActivationFunctionType.Exp
PSUM = bass.MemorySpace.PSUM

B, N, D = 2, 2048, 2048
H, HD = 16, 128
HL = 4            # local heads per core
C = HL * HD       # 512 local head cols
KT = D // 128     # 16 contraction tiles
NB = 4            # n-blocks of 512 for projections
NT = N // 128     # 16 j-tiles
SCALE = float(HD) ** -0.5
N_CORES = 8
PRE = 12          # attention steps pre-run inside the v phase

_CACHE = {}


def _build_program():
    nc = bacc.Bacc("TRN2", target_bir_lowering=False, debug=False,
                   num_devices=N_CORES)

    xt_d = nc.dram_tensor("xt", [NB, 128, KT, 512], BF16, kind="ExternalInput")
    wq_d = nc.dram_tensor("wq", [128, KT, C], BF16, kind="ExternalInput")
    wk_d = nc.dram_tensor("wk", [128, KT, C], BF16, kind="ExternalInput")
    wv_d = nc.dram_tensor("wv", [128, KT, C], BF16, kind="ExternalInput")
    wo_d = nc.dram_tensor("wo", [128, HL, D], BF16, kind="ExternalInput")
    cos_d = nc.dram_tensor("cos", [128, N], BF16, kind="ExternalInput")
    sin_d = nc.dram_tensor("sin", [128, N], BF16, kind="ExternalInput")
    onec_d = nc.dram_tensor("onec", [128, 1], BF16, kind="ExternalInput")
    out_d = nc.dram_tensor("out", [N, D], BF16, kind="ExternalOutput")

    with tile.TileContext(nc) as tc:
        with (
            tc.tile_pool(name="res", bufs=1) as res,
            tc.tile_pool(name="psx", bufs=1, space=PSUM) as psx,
        ):
            qr = res.tile([128, HL, N], BF16)    # q^T per head [d, n]
            kr = res.tile([128, HL, N], BF16)    # k^T per head [d, n]
            vv = res.tile([128, NT, C], BF16)    # v natural [n, c]
            wo_sb = res.tile([128, HL, D], BF16)
            wv_sb = res.tile([128, KT, C], BF16)
            ones_col = res.tile([128, 1], BF16)
            ones_row = res.tile([1, 128], F32)   # for PE partition-broadcast
            # ao lives in the kernel-lifetime pool: a post-p1 pool's alloc
            # would wait on p1's full release barrier and head-block the
            # PE queue right at the attention-main handoff (~8us stall).
            ao = res.tile([128, HL, N], BF16)    # A^T normalized [c, n]

            # the four 2-bank PSUM tags; every PSUM tile in the kernel
            # comes from one of these (bufs=1 rings, rotation by reuse)
            def ps_tile(tg, shape=(128, 1024), name="ps"):
                return psx.tile(list(shape), F32, tag=tg, name=name)

            with tc.tile_pool(name="p1", bufs=1) as p1:
                cos_sb = p1.tile([128, N], BF16, tag="cos")
                sin_sb = p1.tile([128, N], BF16, tag="sin")

                # ---- HAM pre-warm + ACT-table preload during DMA wait
                junk = p1.tile([128, 644], BF16, tag="junk")
                nc.gpsimd.memset(junk[:], 0)
                nc.gpsimd.memset(ones_row[:], 1.0)
                jps = ps_tile("s0", name="jps")
                for _ in range(10):
                    nc.tensor.matmul(jps[:, 0:512], junk[:, 0:128],
                                     junk[:, 128:640], start=True, stop=True)

                # ---- weight prefetch: wk/wq interleaved on the ActE
                # queue; wv after; small tables last.
                wk_sb = p1.tile([128, KT, C], BF16, tag="wk")
                wq_sb = p1.tile([128, KT, C], BF16, tag="wq")
                for t0, t1 in [(0, 2), (2, 4), (4, 8), (8, 16)]:
                    nc.scalar.dma_start(wk_sb[:, t0:t1, :], wk_d[:, t0:t1, :])
                    nc.scalar.dma_start(wq_sb[:, t0:t1, :], wq_d[:, t0:t1, :])
                for t0, t1 in [(0, 8), (8, 16)]:
                    nc.scalar.dma_start(wv_sb[:, t0:t1, :], wv_d[:, t0:t1, :])
                nc.scalar.dma_start(cos_sb[:], cos_d[:])
                nc.scalar.dma_start(sin_sb[:], sin_d[:])
                nc.scalar.dma_start(ones_col[:], onec_d[:])
                # exp table preload (~2.7us ActE-queue stall) AFTER the
                # DMA triggers above, still inside the startup window
                nc.scalar.activation(junk[0:1, 642:643], junk[0:1, 640:641],
                                     EXP)

                def x_halves(nb, first=False):
                    halves = []
                    for hf in range(2):
                        xh = p1.tile([128, 8, 512], BF16, tag="x", bufs=2,
                                     name="xh")
                        lo = hf * 8
                        chunks = ([(0, 2), (2, 4), (4, 8)]
                                  if (first and hf == 0) else [(0, 8)])
                        for c0, c1 in chunks:
                            nc.sync.dma_start(
                                xh[:, c0:c1, :],
                                xt_d[nb][:, lo + c0:lo + c1, :])
                        halves.append(xh)
                    return halves

                def rope_chunk(src, m, nb):
                    # RoPE on a [128, 512] n-chunk of head m, fused right
                    # behind its PSUM drain. t = swap(src)*sin_signed;
                    # src = src*cos + t (sin sign folded in on host).
                    # The d-half swap is a partition shuffle -> SBUF-SBUF
                    # DMA on the sync queue.
                    ns = slice(nb * 512, nb * 512 + 512)
                    sl = src[:, m, ns]
                    tmp = p1.tile([128, 512], BF16, tag="tmp", bufs=2,
                                  name="tmp")
                    # swap triggers on the (otherwise idle) GpSimd queue:
                    # on sync they backlog the x stream, on ActE they delay
                    # the PSUM drains the next block's chains gate on.
                    nc.gpsimd.dma_start(tmp[0:64, :], src[64:128, m, ns])
                    nc.gpsimd.dma_start(tmp[64:128, :], src[0:64, m, ns])
                    nc.vector.tensor_tensor(tmp[:], tmp[:], sin_sb[:, ns],
                                            op=MULT)
                    nc.vector.tensor_tensor(sl, sl, cos_sb[:, ns], op=MULT)
                    nc.vector.tensor_tensor(sl, sl, tmp[:], op=ADD)

                # ---------------- Phase 1: Q/K projections + fused RoPE
                TAGS = ("s0", "s1", "a0", "a1")
                for nb in range(NB):
                    xh = x_halves(nb, first=(nb == 0))
                    pss = [ps_tile(tg, name=f"qk_{tg}") for tg in TAGS]

                    def chain(i):
                        return pss[i // 2][:, (i % 2) * 512:(i % 2) * 512 + 512]

                    for t in range(KT):
                        xs = xh[t // 8][:, t % 8, :]
                        for i, (w_sb, m) in enumerate(
                            (w_sb, m) for w_sb in (wk_sb, wq_sb)
                            for m in range(HL)
                        ):
                            nc.tensor.matmul(
                                chain(i), w_sb[:, t, bass.ts(m, 128)], xs,
                                start=(t == 0), stop=(t == KT - 1),
                            )
                    # drain + rope, head-major so head 0 is ready first
                    for m in range(HL):
                        for ki, dst in ((0, kr), (HL, qr)):
                            nc.scalar.copy(dst[:, m, bass.ts(nb, 512)],
                                           chain(ki + m))
                            rope_chunk(dst, m, nb)

                # ---------------- Phase 2a: V projection + pre-run of the
                # first PRE attention score+exp steps (ActE idle here).
                windows = [(h, ih) for h in range(HL) for ih in range(2)]
                seq = [(w, j) for w in range(len(windows))
                       for j in range(NT)]
                state = {}
                scount = [0]

                def emit_scores(w, j):
                    h, ih = windows[w]
                    s_ps = ps_tile(f"s{scount[0] % 2}", name="s_ps")
                    scount[0] += 1
                    for f in range(2):
                        q0 = ih * 1024 + f * 512
                        nc.tensor.matmul(
                            s_ps[:, bass.ts(f, 512)],
                            kr[:, h, bass.ts(j, 128)],
                            qr[:, h, q0:q0 + 512],
                            start=True, stop=True,
                        )
                    # s_exp tiles are [128, 2048] PAIRS (steps 2p, 2p+1
                    # fill the halves) so denominator accumulation is one
                    # DVE op per pair - DVE at ~90% otherwise starves the
                    # exp stream of s_exp slots.
                    if j % 2 == 0:
                        pair = res.tile([128, 2048], BF16, tag="sexp",
                                        bufs=7, name="s_pair")
                        state[(w, j // 2, "pair")] = pair
                    pair = state[(w, j // 2, "pair")]
                    nc.scalar.activation(
                        pair[:, (j % 2) * 1024:(j % 2) * 1024 + 1024],
                        s_ps[:], EXP, scale=SCALE)

                pre_done = [0]

                def v_pass():
                    for nb in range(NB):
                        xh = x_halves(nb)
                        va = [ps_tile(tg, name=f"v_{tg}")
                              for tg in ("a0", "a1")]
                        for t in range(KT):
                            for m in range(HL):
                                nc.tensor.matmul(
                                    va[m // 2][:, (m % 2) * 512:
                                               (m % 2) * 512 + 512],
                                    xh[t // 8][:, t % 8, bass.ts(m, 128)],
                                    wv_sb[:, t, :],
                                    start=(t == 0), stop=(t == KT - 1),
                                )
                            gt = nb * KT + t
                            if gt % 4 == 3 and pre_done[0] < PRE:
                                emit_scores(*seq[pre_done[0]])
                                pre_done[0] += 1
                        for m in range(HL):
                            # early blocks drain on ActE (the pre-run exp
                            # stream is only ~30% of it); late blocks on
                            # DVE, clear of the fused-rope tail, so the
                            # attention handoff isn't gated on ActE.
                            src_ap = va[m // 2][:, (m % 2) * 512:
                                                (m % 2) * 512 + 512]
                            if nb < 2:
                                nc.scalar.copy(vv[:, nb * HL + m, :], src_ap)
                            else:
                                nc.vector.tensor_copy(vv[:, nb * HL + m, :],
                                                      src_ap)
                        if nb == 0:
                            for hch in range(2):
                                nc.scalar.dma_start(
                                    wo_sb[:, 2 * hch:2 * hch + 2, :],
                                    wo_d[:, 2 * hch:2 * hch + 2, :])

                v_pass()

            # ---------------- Phase 2b: attention main loop -------------
            if True:

                def emit_finish_a(w):
                    # deferred softmax-denominator reduce of window w
                    # (emitted ~2 steps into window w+1): the ones-matmul
                    # waits on the full DVE acc chain, so emitted at the
                    # window end it head-blocks the in-order PE queue.
                    acc_a = state.pop((w, "acca"))
                    l_ps = ps_tile(f"s{scount[0] % 2}", shape=(1, 1024),
                                   name="l_ps")
                    scount[0] += 1
                    for f in range(2):
                        nc.tensor.matmul(
                            l_ps[:, bass.ts(f, 512)],
                            ones_col[:],
                            acc_a[:, f * 512:f * 512 + 512],
                            start=True, stop=False,
                        )
                        nc.tensor.matmul(
                            l_ps[:, bass.ts(f, 512)],
                            ones_col[:],
                            acc_a[:, 1024 + f * 512:1024 + f * 512 + 512],
                            start=False, stop=True,
                        )
                    recip = res.tile([1, 1024], F32, tag="recip", bufs=2,
                                     name="recip")
                    nc.vector.reciprocal_approx_fast(recip[:], l_ps[:])
                    state[w, "recip"] = recip

                def emit_finish_b(w):
                    # deferred normalize+drain of window w's PV accumulator
                    # (emitted ~4 steps into window w+1). The reciprocal
                    # row is partition-broadcast by a K=1 ones-row matmul
                    # on the PE (GpSimd's broadcast thrashes the SBUF port
                    # it shares with DVE), and applied right away so the
                    # s-tag slot is held for under a step.
                    h, ih = windows[w]
                    a_ps = state.pop((w, "a"))
                    recip = state.pop((w, "recip"))
                    bc_ps = ps_tile(f"s{scount[0] % 2}", name="bc_ps")
                    scount[0] += 1
                    for f in range(2):
                        nc.tensor.matmul(
                            bc_ps[:, bass.ts(f, 512)],
                            ones_row[:],
                            recip[:, bass.ts(f, 512)],
                            start=True, stop=True,
                        )
                    # tensor_tensor may read at most ONE input from PSUM:
                    # stage the broadcast through SBUF (fast 4x-mode copy)
                    bc_sb = res.tile([128, 1024], BF16, tag="bcsb", bufs=2,
                                     name="bc_sb")
                    nc.vector.tensor_copy(bc_sb[:], bc_ps[:])
                    nc.vector.tensor_tensor(ao[:, h, bass.ts(ih, 1024)],
                                            a_ps[:], bc_sb[:], op=MULT)

                def emit_consume(w, j):
                    h, ih = windows[w]
                    if j == 0:
                        state[w, "a"] = ps_tile(f"a{w % 2}", name="a_ps")
                    a_ps = state[w, "a"]
                    pair = state[(w, j // 2, "pair")]
                    if j % 2 == 1:
                        state.pop((w, j // 2, "pair"))
                        if j == 1:
                            acc_a = res.tile([128, 2048], BF16, tag="acca",
                                             bufs=2, name="acc_a")
                            nc.vector.tensor_copy(acc_a[:], pair[:])
                            state[w, "acca"] = acc_a
                        else:
                            acc_a = state[w, "acca"]
                            nc.vector.tensor_tensor(acc_a[:], acc_a[:],
                                                    pair[:], op=ADD)
                    for f in range(2):
                        q0 = (j % 2) * 1024 + f * 512
                        nc.tensor.matmul(
                            a_ps[:, bass.ts(f, 512)],
                            vv[:, j, bass.ts(h, 128)],
                            pair[:, q0:q0 + 512],
                            start=(j == 0), stop=(j == NT - 1),
                        )
                    if j == 2 and w > 0:
                        emit_finish_a(w - 1)
                    if j == 6 and w > 0:
                        emit_finish_b(w - 1)

                for idx in range(len(seq)):
                    if idx + PRE < len(seq):
                        emit_scores(*seq[idx + PRE])
                    emit_consume(*seq[idx])
                emit_finish_a(len(windows) - 1)
                emit_finish_b(len(windows) - 1)

            # ---------------- Phase 3: o_proj (bf16) --------------------
            with tc.tile_pool(name="p3", bufs=1) as p3:
                for m in range(NT):
                    st = p3.tile([128, D], BF16, tag="st", bufs=2,
                                 name="st")
                    for f in range(4):
                        o_ps = ps_tile(f"s{f % 2}", shape=(128, 512),
                                       name="o_ps")
                        for ct in range(HL):
                            nc.tensor.matmul(
                                o_ps[:],
                                ao[:, ct, bass.ts(m, 128)],
                                wo_sb[:, ct, bass.ts(f, 512)],
                                start=(ct == 0), stop=(ct == HL - 1),
                            )
                        if f < 3:
                            nc.scalar.copy(st[:, bass.ts(f, 512)], o_ps[:])
                        else:
                            nc.vector.tensor_copy(st[:, bass.ts(f, 512)],
                                                  o_ps[:])
                        if m == NT - 1:
                            # last tile: ship each chunk as it lands so the
                            # final DMA overlaps the remaining copies
                            nc.sync.dma_start(
                                out_d[bass.ts(m, 128), bass.ts(f, 512)],
                                st[:, bass.ts(f, 512)])
                    if m < NT - 1:
                        nc.sync.dma_start(out_d[bass.ts(m, 128), :], st[:])

    nc.compile()
    return nc


def _rope_tables():
    inv_freq = 1.0 / (10000.0 ** (np.arange(0, HD, 2, dtype=np.float32) / HD))
    pos = np.arange(N, dtype=np.float32)
    freqs = pos[:, None] * inv_freq[None, :]          # [N, HD/2]
    emb = np.concatenate([freqs, freqs], axis=-1)     # [N, HD]
    cos = np.cos(emb).astype(np.float32).T.copy()     # [HD, N]
    sin = np.sin(emb).astype(np.float32).T.copy()     # [HD, N]
    sin_signed = sin.copy()
    sin_signed[0:64] *= -1.0
    return cos, sin_signed


def _make_in_maps(x, Wq, Wk, Wv, Wo):
    cos, sin_signed = _rope_tables()
    bf = ml_dtypes.bfloat16

    in_maps = []
    for c in range(N_CORES):
        b, hg = c // 4, c % 4
        cols = slice(C * hg, C * hg + C)
        xT = np.ascontiguousarray(x[b].T)                      # [D, N]
        xt = np.ascontiguousarray(
            xT.reshape(KT, 128, NB, 512).transpose(2, 1, 0, 3)
        ).astype(bf)                                           # [NB,128,KT,512]

        def wslice(W):
            wt = W[cols, :].T                                  # [D, C]
            return np.ascontiguousarray(
                wt.reshape(KT, 128, C).transpose(1, 0, 2)
            ).astype(bf)                                       # [128, KT, C]

        wo_t = Wo[:, cols].T                                   # [C, D]
        wo = np.ascontiguousarray(
            wo_t.reshape(HL, 128, D).transpose(1, 0, 2)
        ).astype(bf)                                           # [128, HL, D]

        in_maps.append({
            "xt": xt,
            "wq": wslice(Wq),
            "wk": wslice(Wk),
            "wv": wslice(Wv),
            "wo": wo,
            "cos": cos.astype(bf),
            "sin": sin_signed.astype(bf),
            "onec": np.ones((128, 1), dtype=bf),
        })
    return in_maps


def kernel(x, Wq, Wk, Wv, Wo):
    x = np.asarray(x, dtype=np.float32)
    Wq = np.asarray(Wq, dtype=np.float32)
    Wk = np.asarray(Wk, dtype=np.float32)
    Wv = np.asarray(Wv, dtype=np.float32)
    Wo = np.asarray(Wo, dtype=np.float32)

    if "nc" not in _CACHE:
        _CACHE["nc"] = _build_program()
    nc = _CACHE["nc"]

    in_maps = _make_in_maps(x, Wq, Wk, Wv, Wo)
    results = run_bass_kernel_spmd(
        nc, in_maps, core_ids=list(range(N_CORES))
    ).results

    out = np.zeros((B, N, D), dtype=np.float32)
    for c in range(N_CORES):
        out[c // 4] += np.asarray(results[c]["out"], dtype=np.float32)
    return out


# revision 43
# speedup vs baseline: 1.0612x; 1.0612x over previous
"""MHSA + RoPE kernel for Trainium2, 8 NeuronCores.

Sharding: data-parallel over batch (B=2) x tensor-parallel over heads
(16 heads -> 4 head-groups of 4). Core c handles batch c//4, heads
[4*(c%4) : 4*(c%4)+4]. Each core computes its partial o_proj output
[N, D]; host sums the 4 partials per batch (the "all-reduce").

Design (arrived at via NTFF traces; 778us baseline -> ~395us):
  - Everything on the PE runs bf16: HW runs f32r matmuls ~2x slower
    per row than bf16 despite the cost model's parity claim, and bf16
    also cuts power throttling (467us@67% -> ~20us@92% util limit).
  - Projections: one merged q+k pass (8 interleaved PSUM chains,
    t-outer so the first matmul starts after one small DMA chunk),
    then a v pass; x streams twice. RoPE (bf16, DVE + SBUF-SBUF swap
    DMAs) runs entirely under the v pass. Input DMas are split across
    the two HWDGE queues (SP for x/out, Activation for weights/swaps).
  - Attention: one software-pipelined stream over all (head, q-half,
    j) steps; the scores+exp stream runs 2 steps ahead of the
    accumulate+PV stream ACROSS window boundaries, so the in-order PE
    queue never waits on exp and ActE (the pacer: 128 exps of
    [128,1024] at ~1.05us) streams gap-free. PSUM: 2x2-bank scores
    ring + 2x2-bank PV-accumulator ring = all 8 banks.
  - Softmax denominators: exp tiles accumulate in bf16 on VectorE
    (16 sequential adds -> exact f32 ones-matmul partition-reduce ->
    reciprocal_approx_fast -> partition_broadcast on GpSimd), applied
    to the PV accumulator on VectorE. GpSimd cannot touch PSUM.
  - o_proj in bf16 with per-chunk output DMA on the last tile.
Output is bf16; the host sums the 4 TP partials per batch in f32.
"""

import sys

sys.path.insert(0, "/opt/trn_rl_repo")

import numpy as np
import ml_dtypes

import concourse.bass as bass
import concourse.tile as tile
from concourse import bacc, mybir
from concourse.bass_utils import run_bass_kernel_spmd

F32 = mybir.dt.float32
F32R = mybir.dt.float32r
BF16 = mybir.dt.bfloat16
MULT = mybir.AluOpType.mult
ADD = mybir.AluOpType.add
EXP = mybir.ActivationFunctionType.Exp
PSUM = bass.MemorySpace.PSUM

B, N, D = 2, 2048, 2048
H, HD = 16, 128
HL = 4            # local heads per core
C = HL * HD       # 512 local head cols
KT = D // 128     # 16 contraction tiles
NB = 4            # n-blocks of 512 for projections
NT = N // 128     # 16 j-tiles
SCALE = float(HD) ** -0.5
N_CORES = 8

_CACHE = {}


def _build_program():
    nc = bacc.Bacc("TRN2", target_bir_lowering=False, debug=False,
                   num_devices=N_CORES)

    xt_d = nc.dram_tensor("xt", [NB, 128, KT, 512], BF16, kind="ExternalInput")
    wq_d = nc.dram_tensor("wq", [128, KT, C], BF16, kind="ExternalInput")
    wk_d = nc.dram_tensor("wk", [128, KT, C], BF16, kind="ExternalInput")
    wv_d = nc.dram_tensor("wv", [128, KT, C], BF16, kind="ExternalInput")
    wo_d = nc.dram_tensor("wo", [128, HL, D], BF16, kind="ExternalInput")
    cos_d = nc.dram_tensor("cos", [128, N], BF16, kind="ExternalInput")
    sin_d = nc.dram_tensor("sin", [128, N], BF16, kind="ExternalInput")
    onec_d = nc.dram_tensor("onec", [128, 1], BF16, kind="ExternalInput")
    out_d = nc.dram_tensor("out", [N, D], BF16, kind="ExternalOutput")

    with tile.TileContext(nc) as tc:
        with tc.tile_pool(name="res", bufs=1) as res:
            qr = res.tile([128, HL, N], BF16)    # q^T per head [d, n]
            kr = res.tile([128, HL, N], BF16)    # k^T per head [d, n]
            vv = res.tile([128, NT, C], BF16)    # v natural [n, c]
            ao = res.tile([128, HL, N], BF16)    # A^T normalized [c, n]
            wo_sb = res.tile([128, HL, D], BF16)
            cos_sb = res.tile([128, N], BF16)
            sin_sb = res.tile([128, N], BF16)
            ones_col = res.tile([128, 1], BF16)

            # ---------------- Phase 1: Q/K/V projections (bf16) ---------
            # q+k in one pass over x (8 psum chains), then v in a second
            # pass. x chunks stream on the SP DMA queue; weights + RoPE
            # swaps ride the Activation HWDGE queue so the two overlap.
            with (
                tc.tile_pool(name="p1", bufs=1) as p1,
                tc.tile_pool(name="ps1", bufs=1, space=PSUM) as ps1,
            ):
                w_sbs = {}
                wchunks = {"wk": [(0, 2), (2, 4), (4, 8), (8, 16)],
                           "wq": [(0, 2), (2, 8), (8, 16)],
                           "wv": [(0, 8), (8, 16)]}
                for wd, wname in ((wk_d, "wk"), (wq_d, "wq"), (wv_d, "wv")):
                    w_sb = p1.tile([128, KT, C], BF16, tag=f"w_{wname}")
                    for t0, t1 in wchunks[wname]:
                        nc.scalar.dma_start(w_sb[:, t0:t1, :], wd[:, t0:t1, :])
                    w_sbs[wname] = w_sb

                def qk_pass():
                    for nb in range(NB):
                        x_sb = p1.tile([128, KT, 512], BF16, tag="x", bufs=2)
                        chunks = ([(0, 2), (2, 4), (4, 8), (8, 16)]
                                  if nb == 0 else [(0, 8), (8, 16)])
                        for t0, t1 in chunks:
                            nc.sync.dma_start(x_sb[:, t0:t1, :],
                                              xt_d[nb][:, t0:t1, :])
                        if nb == 0:
                            # small tables: not needed until RoPE/attention;
                            # queued behind the first x block they keep the
                            # SP queue streaming without delaying it.
                            nc.sync.dma_start(ones_col[:], onec_d[:])
                            nc.sync.dma_start(cos_sb[:], cos_d[:])
                            nc.sync.dma_start(sin_sb[:], sin_d[:])
                        pss = [ps1.tile([128, 512], F32, tag=f"pp{i}",
                                        name=f"pp{i}")
                               for i in range(2 * HL)]
                        for t in range(KT):
                            for i, (w_sb, m) in enumerate(
                                (w_sbs[w], m) for w in ("wk", "wq")
                                for m in range(HL)
                            ):
                                nc.tensor.matmul(
                                    pss[i], w_sb[:, t, bass.ts(m, 128)],
                                    x_sb[:, t, :],
                                    start=(t == 0), stop=(t == KT - 1),
                                )
                        for i, (dst, m) in enumerate(
                            (dst, m) for dst in (kr, qr) for m in range(HL)
                        ):
                            nc.scalar.copy(dst[:, m, bass.ts(nb, 512)],
                                           pss[i])

                def v_pass():
                    w_sb = w_sbs["wv"]
                    for nb in range(NB):
                        x_sb = p1.tile([128, KT, 512], BF16, tag="x", bufs=2)
                        for tch in range(2):
                            tsl = slice(8 * tch, 8 * tch + 8)
                            nc.sync.dma_start(x_sb[:, tsl, :],
                                              xt_d[nb][:, tsl, :])
                        pss = [ps1.tile([128, 512], F32,
                                        tag=f"pp{(nb % 2) * HL + m}",
                                        name=f"pp{m}")
                               for m in range(HL)]
                        for t in range(KT):
                            for m in range(HL):
                                nc.tensor.matmul(
                                    pss[m], x_sb[:, t, bass.ts(m, 128)],
                                    w_sb[:, t, :],
                                    start=(t == 0), stop=(t == KT - 1),
                                )
                        for m in range(HL):
                            nc.scalar.copy(vv[:, nb * HL + m, :], pss[m])
                        if nb == 0:
                            for hch in range(2):
                                nc.scalar.dma_start(
                                    wo_sb[:, 2 * hch:2 * hch + 2, :],
                                    wo_d[:, 2 * hch:2 * hch + 2, :])

                def rope(src):
                    # t = shift(src) * sin_signed; src *= cos; src += t
                    # (sign of sin folded in on host). The d-half swap is a
                    # partition shuffle - done with SBUF->SBUF DMA.
                    for h in range(HL):
                        sl = src[:, h, :]
                        tmp = p1.tile([128, N], BF16, tag="tmp", bufs=2)
                        nc.scalar.dma_start(tmp[0:64, :], sl[64:128, :])
                        nc.scalar.dma_start(tmp[64:128, :], sl[0:64, :])
                        nc.vector.tensor_tensor(tmp[:], tmp[:], sin_sb[:],
                                                op=MULT)
                        nc.vector.tensor_tensor(sl, sl, cos_sb[:], op=MULT)
                        nc.vector.tensor_tensor(sl, sl, tmp[:], op=ADD)

                qk_pass()
                rope(kr)          # DVE + DMA, overlaps v pass on PE
                rope(qr)
                v_pass()

            # ---------------- Phase 2: RoPE'd attention (bf16) ----------
            with (
                tc.tile_pool(name="p2", bufs=1) as p2,
                tc.tile_pool(name="ps_s", bufs=2, space=PSUM) as ps_s,
                tc.tile_pool(name="ps_a", bufs=2, space=PSUM) as ps_a,
            ):
                # One software-pipelined stream over all (head, ih-half,
                # j) steps: the scores/exp stream runs LOOKAHEAD steps
                # ahead of the PV/accumulate stream, across window
                # boundaries, so neither PE nor ActE ever re-syncs at a
                # window edge. acc stays bf16 end-to-end (only 16
                # sequential bf16 adds per partition before the exact f32
                # matmul reduction -> denominator error ~1e-3).
                windows = [(h, ih) for h in range(HL) for ih in range(2)]
                seq = [(w, j) for w in range(len(windows)) for j in range(NT)]
                LOOKAHEAD = 2
                state = {}

                def emit_scores(w, j):
                    h, ih = windows[w]
                    s_ps = ps_s.tile([128, 1024], F32, tag="s", name="s_ps")
                    for f in range(2):
                        q0 = ih * 1024 + f * 512
                        nc.tensor.matmul(
                            s_ps[:, bass.ts(f, 512)],
                            kr[:, h, bass.ts(j, 128)],
                            qr[:, h, q0:q0 + 512],
                            start=True, stop=True,
                        )
                    s_exp = p2.tile([128, 1024], BF16, tag="sexp",
                                    bufs=6, name="s_exp")
                    nc.scalar.activation(s_exp[:], s_ps[:], EXP, scale=SCALE)
                    state[(w, j)] = s_exp

                def emit_consume(w, j):
                    h, ih = windows[w]
                    if j == 0:
                        state[w, "a"] = ps_a.tile([128, 1024], F32, tag="a",
                                                  name="a_ps")
                        acc = p2.tile([128, 1024], BF16, tag="acc", bufs=2,
                                      name="acc")
                        nc.gpsimd.memset(acc[:], 0)
                        state[w, "acc"] = acc
                    a_ps, acc = state[w, "a"], state[w, "acc"]
                    s_exp = state.pop((w, j))
                    nc.vector.tensor_tensor(acc[:], acc[:], s_exp[:], op=ADD)
                    for f in range(2):
                        nc.tensor.matmul(
                            a_ps[:, bass.ts(f, 512)],
                            vv[:, j, bass.ts(h, 128)],
                            s_exp[:, bass.ts(f, 512)],
                            start=(j == 0), stop=(j == NT - 1),
                        )
                    if j == NT - 1:
                        # softmax denominators: partition-reduce acc with a
                        # ones-column matmul (exact f32 in PSUM), fast
                        # reciprocal on VectorE, broadcast over partitions
                        # on GpSimd, apply on VectorE.
                        l_ps = ps_s.tile([1, 1024], F32, tag="s")
                        for f in range(2):
                            nc.tensor.matmul(
                                l_ps[:, bass.ts(f, 512)],
                                ones_col[:],
                                acc[:, bass.ts(f, 512)],
                                start=True, stop=True,
                            )
                        recip = p2.tile([1, 1024], F32, tag="recip", bufs=2)
                        nc.vector.reciprocal_approx_fast(recip[:], l_ps[:])
                        bc_sb = p2.tile([128, 1024], F32, tag="bcsb", bufs=2)
                        nc.gpsimd.partition_broadcast(bc_sb[:],
                                                      recip[0:1, :])
                        nc.vector.tensor_tensor(ao[:, h, bass.ts(ih, 1024)],
                                                a_ps[:], bc_sb[:], op=MULT)

                for idx in range(len(seq) + LOOKAHEAD):
                    if idx < len(seq):
                        emit_scores(*seq[idx])
                    if idx >= LOOKAHEAD:
                        emit_consume(*seq[idx - LOOKAHEAD])

            # ---------------- Phase 3: o_proj (bf16) --------------------
            with (
                tc.tile_pool(name="p3", bufs=1) as p3,
                tc.tile_pool(name="ps3", bufs=2, space=PSUM) as ps3,
            ):
                for m in range(NT):
                    st = p3.tile([128, D], BF16, tag="st", bufs=2)
                    for f in range(4):
                        o_ps = ps3.tile([128, 512], F32, tag=f"o{f % 2}")
                        for ct in range(HL):
                            nc.tensor.matmul(
                                o_ps[:],
                                ao[:, ct, bass.ts(m, 128)],
                                wo_sb[:, ct, bass.ts(f, 512)],
                                start=(ct == 0), stop=(ct == HL - 1),
                            )
                        if f < 3:
                            nc.scalar.copy(st[:, bass.ts(f, 512)], o_ps[:])
                        else:
                            nc.vector.tensor_copy(st[:, bass.ts(f, 512)],
                                                  o_ps[:])
                        if m == NT - 1:
                            # last tile: ship each chunk as it lands so the
                            # final DMA overlaps the remaining copies
                            nc.sync.dma_start(
                                out_d[bass.ts(m, 128), bass.ts(f, 512)],
                                st[:, bass.ts(f, 512)])
                    if m < NT - 1:
                        nc.sync.dma_start(out_d[bass.ts(m, 128), :], st[:])

    nc.compile()
    return nc


def _rope_tables():
    inv_freq = 1.0 / (10000.0 ** (np.arange(0, HD, 2, dtype=np.float32) / HD))
    pos = np.arange(N, dtype=np.float32)
    freqs = pos[:, None] * inv_freq[None, :]          # [N, HD/2]
    emb = np.concatenate([freqs, freqs], axis=-1)     # [N, HD]
    cos = np.cos(emb).astype(np.float32).T.copy()     # [HD, N]
    sin = np.sin(emb).astype(np.float32).T.copy()     # [HD, N]
    sin_signed = sin.copy()
    sin_signed[0:64] *= -1.0
    return cos, sin_signed


def _make_in_maps(x, Wq, Wk, Wv, Wo):
    cos, sin_signed = _rope_tables()
    bf = ml_dtypes.bfloat16

    in_maps = []
    for c in range(N_CORES):
        b, hg = c // 4, c % 4
        cols = slice(C * hg, C * hg + C)
        xT = np.ascontiguousarray(x[b].T)                      # [D, N]
        xt = np.ascontiguousarray(
            xT.reshape(KT, 128, NB, 512).transpose(2, 1, 0, 3)
        ).astype(bf)                                           # [NB,128,KT,512]

        def wslice(W):
            wt = W[cols, :].T                                  # [D, C]
            return np.ascontiguousarray(
                wt.reshape(KT, 128, C).transpose(1, 0, 2)
            ).astype(bf)                                       # [128, KT, C]

        wo_t = Wo[:, cols].T                                   # [C, D]
        wo = np.ascontiguousarray(
            wo_t.reshape(HL, 128, D).transpose(1, 0, 2)
        ).astype(bf)                                           # [128, HL, D]

        in_maps.append({
            "xt": xt,
            "wq": wslice(Wq),
            "wk": wslice(Wk),
            "wv": wslice(Wv),
            "wo": wo,
            "cos": cos.astype(bf),
            "sin": sin_signed.astype(bf),
            "onec": np.ones((128, 1), dtype=bf),
        })
    return in_maps


def kernel(x, Wq, Wk, Wv, Wo):
    x = np.asarray(x, dtype=np.float32)
    Wq = np.asarray(Wq, dtype=np.float32)
    Wk = np.asarray(Wk, dtype=np.float32)
    Wv = np.asarray(Wv, dtype=np.float32)
    Wo = np.asarray(Wo, dtype=np.float32)

    if "nc" not in _CACHE:
        _CACHE["nc"] = _build_program()
    nc = _CACHE["nc"]

    in_maps = _make_in_maps(x, Wq, Wk, Wv, Wo)
    results = run_bass_kernel_spmd(
        nc, in_maps, core_ids=list(range(N_CORES))
    ).results

    out = np.zeros((B, N, D), dtype=np.float32)
    for c in range(N_CORES):
        out[c // 4] += np.asarray(results[c]["out"], dtype=np.float32)
    return out



# revision 44
# speedup vs baseline: 1.0709x; 1.0091x over previous
"""MHSA + RoPE kernel for Trainium2, 8 NeuronCores.

Sharding: data-parallel over batch (B=2) x tensor-parallel over heads
(16 heads -> 4 head-groups of 4). Core c handles batch c//4, heads
[4*(c%4) : 4*(c%4)+4]. Each core computes its partial o_proj output
[N, D]; host sums the 4 partials per batch (the "all-reduce").

Design (arrived at via NTFF traces; 778us baseline -> ~395us):
  - Everything on the PE runs bf16: HW runs f32r matmuls ~2x slower
    per row than bf16 despite the cost model's parity claim, and bf16
    also cuts power throttling (467us@67% -> ~20us@92% util limit).
  - Projections: one merged q+k pass (8 interleaved PSUM chains,
    t-outer so the first matmul starts after one small DMA chunk),
    then a v pass; x streams twice. RoPE (bf16, DVE + SBUF-SBUF swap
    DMAs) runs entirely under the v pass. Input DMas are split across
    the two HWDGE queues (SP for x/out, Activation for weights/swaps).
  - Attention: one software-pipelined stream over all (head, q-half,
    j) steps; the scores+exp stream runs 2 steps ahead of the
    accumulate+PV stream ACROSS window boundaries, so the in-order PE
    queue never waits on exp and ActE (the pacer: 128 exps of
    [128,1024] at ~1.05us) streams gap-free. PSUM: 2x2-bank scores
    ring + 2x2-bank PV-accumulator ring = all 8 banks.
  - Softmax denominators: exp tiles accumulate in bf16 on VectorE
    (16 sequential adds -> exact f32 ones-matmul partition-reduce ->
    reciprocal_approx_fast -> partition_broadcast on GpSimd), applied
    to the PV accumulator on VectorE. GpSimd cannot touch PSUM.
  - o_proj in bf16 with per-chunk output DMA on the last tile.
Output is bf16; the host sums the 4 TP partials per batch in f32.
"""

import sys

sys.path.insert(0, "/opt/trn_rl_repo")

import numpy as np
import ml_dtypes

import concourse.bass as bass
import concourse.tile as tile
from concourse import bacc, mybir
from concourse.bass_utils import run_bass_kernel_spmd

F32 = mybir.dt.float32
F32R = mybir.dt.float32r
BF16 = mybir.dt.bfloat16
MULT = mybir.AluOpType.mult
ADD = mybir.AluOpType.add
EXP = mybir.ActivationFunctionType.Exp
PSUM = bass.MemorySpace.PSUM

B, N, D = 2, 2048, 2048
H, HD = 16, 128
HL = 4            # local heads per core
C = HL * HD       # 512 local head cols
KT = D // 128     # 16 contraction tiles
NB = 4            # n-blocks of 512 for projections
NT = N // 128     # 16 j-tiles
SCALE = float(HD) ** -0.5
N_CORES = 8

_CACHE = {}


def _build_program():
    nc = bacc.Bacc("TRN2", target_bir_lowering=False, debug=False,
                   num_devices=N_CORES)

    xt_d = nc.dram_tensor("xt", [NB, 128, KT, 512], BF16, kind="ExternalInput")
    wq_d = nc.dram_tensor("wq", [128, KT, C], BF16, kind="ExternalInput")
    wk_d = nc.dram_tensor("wk", [128, KT, C], BF16, kind="ExternalInput")
    wv_d = nc.dram_tensor("wv", [128, KT, C], BF16, kind="ExternalInput")
    wo_d = nc.dram_tensor("wo", [128, HL, D], BF16, kind="ExternalInput")
    cos_d = nc.dram_tensor("cos", [128, N], BF16, kind="ExternalInput")
    sin_d = nc.dram_tensor("sin", [128, N], BF16, kind="ExternalInput")
    onec_d = nc.dram_tensor("onec", [128, 1], BF16, kind="ExternalInput")
    out_d = nc.dram_tensor("out", [N, D], BF16, kind="ExternalOutput")

    with tile.TileContext(nc) as tc:
        with tc.tile_pool(name="res", bufs=1) as res:
            qr = res.tile([128, HL, N], BF16)    # q^T per head [d, n]
            kr = res.tile([128, HL, N], BF16)    # k^T per head [d, n]
            vv = res.tile([128, NT, C], BF16)    # v natural [n, c]
            ao = res.tile([128, HL, N], BF16)    # A^T normalized [c, n]
            wo_sb = res.tile([128, HL, D], BF16)
            cos_sb = res.tile([128, N], BF16)
            sin_sb = res.tile([128, N], BF16)
            ones_col = res.tile([128, 1], BF16)

            # ---------------- Phase 1: Q/K/V projections (bf16) ---------
            # q+k in one pass over x (8 psum chains), then v in a second
            # pass. x chunks stream on the SP DMA queue; weights + RoPE
            # swaps ride the Activation HWDGE queue so the two overlap.
            with (
                tc.tile_pool(name="p1", bufs=1) as p1,
                tc.tile_pool(name="ps1", bufs=1, space=PSUM) as ps1,
            ):
                # HAM pre-warm: dummy matmuls fill the otherwise-idle DMA
                # wait at kernel start so the PE clock gate is already 8/8
                # when the first real matmul issues (saves the ~3.4us
                # cold-clock ramp). Shares pool/tags with the real chains so
                # no extra pool-close barrier is emitted.
                junk = p1.tile([128, 644], BF16, tag="junk")
                nc.gpsimd.memset(junk[:], 0)
                jps = ps1.tile([128, 512], F32, tag="pp0", name="jps")
                for _ in range(10):
                    nc.tensor.matmul(jps[:], junk[:, 0:128], junk[:, 128:640],
                                     start=True, stop=True)

                # wk and wq interleaved on the Activation queue so wq's early
                # t-chunks land in lockstep with PE consumption instead of
                # queued behind all of wk. wv is only needed at v_pass.
                w_sbs = {}
                for wd, wname in ((wk_d, "wk"), (wq_d, "wq"), (wv_d, "wv")):
                    w_sbs[wname] = p1.tile([128, KT, C], BF16,
                                           tag=f"w_{wname}", name=wname)
                for t0, t1 in [(0, 2), (2, 4), (4, 8), (8, 16)]:
                    nc.scalar.dma_start(w_sbs["wk"][:, t0:t1, :],
                                        wk_d[:, t0:t1, :])
                    nc.scalar.dma_start(w_sbs["wq"][:, t0:t1, :],
                                        wq_d[:, t0:t1, :])
                for t0, t1 in [(0, 8), (8, 16)]:
                    nc.scalar.dma_start(w_sbs["wv"][:, t0:t1, :],
                                        wv_d[:, t0:t1, :])
                # preload the exp activation table set (~2.7us ActE-queue
                # stall) inside the startup DMA-wait window - but AFTER the
                # weight-DMA trigger instructions above, which share the
                # ActE queue. (cols 640+ are outside the dummy-mm operands)
                nc.scalar.activation(junk[0:1, 642:643], junk[0:1, 640:641],
                                     EXP)

                def qk_pass():
                    for nb in range(NB):
                        x_sb = p1.tile([128, KT, 512], BF16, tag="x", bufs=2)
                        chunks = ([(0, 2), (2, 4), (4, 8), (8, 16)]
                                  if nb == 0 else [(0, 8), (8, 16)])
                        for t0, t1 in chunks:
                            nc.sync.dma_start(x_sb[:, t0:t1, :],
                                              xt_d[nb][:, t0:t1, :])
                        if nb == 0:
                            # small tables: not needed until RoPE/attention;
                            # queued behind the first x block they keep the
                            # SP queue streaming without delaying it.
                            nc.sync.dma_start(ones_col[:], onec_d[:])
                            nc.sync.dma_start(cos_sb[:], cos_d[:])
                            nc.sync.dma_start(sin_sb[:], sin_d[:])
                        pss = [ps1.tile([128, 512], F32, tag=f"pp{i}",
                                        name=f"pp{i}")
                               for i in range(2 * HL)]
                        for t in range(KT):
                            for i, (w_sb, m) in enumerate(
                                (w_sbs[w], m) for w in ("wk", "wq")
                                for m in range(HL)
                            ):
                                nc.tensor.matmul(
                                    pss[i], w_sb[:, t, bass.ts(m, 128)],
                                    x_sb[:, t, :],
                                    start=(t == 0), stop=(t == KT - 1),
                                )
                        for i, (dst, m) in enumerate(
                            (dst, m) for dst in (kr, qr) for m in range(HL)
                        ):
                            nc.scalar.copy(dst[:, m, bass.ts(nb, 512)],
                                           pss[i])

                def v_pass():
                    w_sb = w_sbs["wv"]
                    for nb in range(NB):
                        x_sb = p1.tile([128, KT, 512], BF16, tag="x", bufs=2)
                        for tch in range(2):
                            tsl = slice(8 * tch, 8 * tch + 8)
                            nc.sync.dma_start(x_sb[:, tsl, :],
                                              xt_d[nb][:, tsl, :])
                        pss = [ps1.tile([128, 512], F32,
                                        tag=f"pp{(nb % 2) * HL + m}",
                                        name=f"pp{m}")
                               for m in range(HL)]
                        for t in range(KT):
                            for m in range(HL):
                                nc.tensor.matmul(
                                    pss[m], x_sb[:, t, bass.ts(m, 128)],
                                    w_sb[:, t, :],
                                    start=(t == 0), stop=(t == KT - 1),
                                )
                        for m in range(HL):
                            # last block's drains on DVE: the first attention
                            # matmul gates on them and ActE serializes them
                            # ~0.7us apiece behind its DMA triggers.
                            if nb == NB - 1:
                                nc.vector.tensor_copy(vv[:, nb * HL + m, :],
                                                      pss[m])
                            else:
                                nc.scalar.copy(vv[:, nb * HL + m, :], pss[m])
                        if nb == 0:
                            for hch in range(2):
                                nc.scalar.dma_start(
                                    wo_sb[:, 2 * hch:2 * hch + 2, :],
                                    wo_d[:, 2 * hch:2 * hch + 2, :])

                def rope(src):
                    # t = shift(src) * sin_signed; src *= cos; src += t
                    # (sign of sin folded in on host). The d-half swap is a
                    # partition shuffle - done with SBUF->SBUF DMA.
                    for h in range(HL):
                        sl = src[:, h, :]
                        tmp = p1.tile([128, N], BF16, tag="tmp", bufs=2)
                        nc.scalar.dma_start(tmp[0:64, :], sl[64:128, :])
                        nc.scalar.dma_start(tmp[64:128, :], sl[0:64, :])
                        nc.vector.tensor_tensor(tmp[:], tmp[:], sin_sb[:],
                                                op=MULT)
                        nc.vector.tensor_tensor(sl, sl, cos_sb[:], op=MULT)
                        nc.vector.tensor_tensor(sl, sl, tmp[:], op=ADD)

                qk_pass()
                rope(kr)          # DVE + DMA, overlaps v pass on PE
                rope(qr)
                v_pass()

            # ---------------- Phase 2: RoPE'd attention (bf16) ----------
            with (
                tc.tile_pool(name="p2", bufs=1) as p2,
                tc.tile_pool(name="ps_s", bufs=2, space=PSUM) as ps_s,
                tc.tile_pool(name="ps_a", bufs=2, space=PSUM) as ps_a,
            ):
                # One software-pipelined stream over all (head, ih-half,
                # j) steps: the scores/exp stream runs LOOKAHEAD steps
                # ahead of the PV/accumulate stream, across window
                # boundaries, so neither PE nor ActE ever re-syncs at a
                # window edge. acc stays bf16 end-to-end (only 16
                # sequential bf16 adds per partition before the exact f32
                # matmul reduction -> denominator error ~1e-3).
                windows = [(h, ih) for h in range(HL) for ih in range(2)]
                seq = [(w, j) for w in range(len(windows)) for j in range(NT)]
                LOOKAHEAD = 2
                state = {}

                def emit_scores(w, j):
                    h, ih = windows[w]
                    s_ps = ps_s.tile([128, 1024], F32, tag="s", name="s_ps")
                    for f in range(2):
                        q0 = ih * 1024 + f * 512
                        nc.tensor.matmul(
                            s_ps[:, bass.ts(f, 512)],
                            kr[:, h, bass.ts(j, 128)],
                            qr[:, h, q0:q0 + 512],
                            start=True, stop=True,
                        )
                    # s_exp tiles are [128, 2048] PAIRS (steps 2p, 2p+1 fill
                    # the two halves) so the denominator accumulation below
                    # is one DVE op per pair instead of per step - DVE at
                    # ~90% starves the exp stream of s_exp slots otherwise.
                    if j % 2 == 0:
                        pair = p2.tile([128, 2048], BF16, tag="sexp",
                                       bufs=5, name="s_pair")
                        state[(w, j // 2, "pair")] = pair
                    pair = state[(w, j // 2, "pair")]
                    nc.scalar.activation(
                        pair[:, (j % 2) * 1024:(j % 2) * 1024 + 1024],
                        s_ps[:], EXP, scale=SCALE)

                def emit_finish_a(w):
                    # deferred softmax-denominator reduce of window w,
                    # emitted ~2 steps into window w+1: the ones-matmul
                    # waits on the full DVE acc chain, so emitted in-stream
                    # at the window end it head-blocks the in-order PE queue
                    # (measured 5.7us wedge + HAM re-throttle).
                    acc_a = state.pop((w, "acca"))
                    l_ps = ps_s.tile([1, 1024], F32, tag="s")
                    for f in range(2):
                        # reduce both halves (even-j and odd-j partial sums)
                        nc.tensor.matmul(
                            l_ps[:, bass.ts(f, 512)],
                            ones_col[:],
                            acc_a[:, f * 512:f * 512 + 512],
                            start=True, stop=False,
                        )
                        nc.tensor.matmul(
                            l_ps[:, bass.ts(f, 512)],
                            ones_col[:],
                            acc_a[:, 1024 + f * 512:1024 + f * 512 + 512],
                            start=False, stop=True,
                        )
                    recip = p2.tile([1, 1024], F32, tag="recip", bufs=2)
                    nc.vector.reciprocal_approx_fast(recip[:], l_ps[:])
                    bc_sb = p2.tile([128, 1024], F32, tag="bcsb", bufs=2)
                    nc.gpsimd.partition_broadcast(bc_sb[:], recip[0:1, :])
                    state[w, "bc"] = bc_sb

                def emit_finish_b(w):
                    # deferred normalize+drain of window w's PV accumulator,
                    # emitted ~6 steps into window w+1 so its wait on the
                    # GpSimd broadcast doesn't head-block the strict-FIFO
                    # DVE queue (which must keep draining acc-adds to free
                    # s_exp slots for the exp stream).
                    h, ih = windows[w]
                    a_ps = state.pop((w, "a"))
                    bc_sb = state.pop((w, "bc"))
                    nc.vector.tensor_tensor(ao[:, h, bass.ts(ih, 1024)],
                                            a_ps[:], bc_sb[:], op=MULT)

                def emit_consume(w, j):
                    h, ih = windows[w]
                    if j == 0:
                        state[w, "a"] = ps_a.tile([128, 1024], F32, tag="a",
                                                  name="a_ps")
                    a_ps = state[w, "a"]
                    pair = state[(w, j // 2, "pair")]
                    # denominator accumulation per PAIR: acc_a is [128,2048]
                    # whose halves hold even-j and odd-j partial sums (the
                    # ones-matmul later reduces both). First pair is a copy
                    # (no memset - keeps GpSimd off the critical path).
                    if j % 2 == 1:
                        state.pop((w, j // 2, "pair"))
                        if j == 1:
                            acc_a = p2.tile([128, 2048], BF16, tag="acca",
                                            bufs=2, name="acc_a")
                            nc.vector.tensor_copy(acc_a[:], pair[:])
                            state[w, "acca"] = acc_a
                        else:
                            acc_a = state[w, "acca"]
                            nc.vector.tensor_tensor(acc_a[:], acc_a[:],
                                                    pair[:], op=ADD)
                    for f in range(2):
                        q0 = (j % 2) * 1024 + f * 512
                        nc.tensor.matmul(
                            a_ps[:, bass.ts(f, 512)],
                            vv[:, j, bass.ts(h, 128)],
                            pair[:, q0:q0 + 512],
                            start=(j == 0), stop=(j == NT - 1),
                        )
                    if j == 2 and w > 0:
                        emit_finish_a(w - 1)
                    if j == 6 and w > 0:
                        emit_finish_b(w - 1)

                for idx in range(len(seq) + LOOKAHEAD):
                    if idx < len(seq):
                        emit_scores(*seq[idx])
                    if idx >= LOOKAHEAD:
                        emit_consume(*seq[idx - LOOKAHEAD])
                emit_finish_a(len(windows) - 1)
                emit_finish_b(len(windows) - 1)

            # ---------------- Phase 3: o_proj (bf16) --------------------
            with (
                tc.tile_pool(name="p3", bufs=1) as p3,
                tc.tile_pool(name="ps3", bufs=2, space=PSUM) as ps3,
            ):
                for m in range(NT):
                    st = p3.tile([128, D], BF16, tag="st", bufs=2)
                    for f in range(4):
                        o_ps = ps3.tile([128, 512], F32, tag=f"o{f % 2}")
                        for ct in range(HL):
                            nc.tensor.matmul(
                                o_ps[:],
                                ao[:, ct, bass.ts(m, 128)],
                                wo_sb[:, ct, bass.ts(f, 512)],
                                start=(ct == 0), stop=(ct == HL - 1),
                            )
                        if f < 3:
                            nc.scalar.copy(st[:, bass.ts(f, 512)], o_ps[:])
                        else:
                            nc.vector.tensor_copy(st[:, bass.ts(f, 512)],
                                                  o_ps[:])
                        if m == NT - 1:
                            # last tile: ship each chunk as it lands so the
                            # final DMA overlaps the remaining copies
                            nc.sync.dma_start(
                                out_d[bass.ts(m, 128), bass.ts(f, 512)],
                                st[:, bass.ts(f, 512)])
                    if m < NT - 1:
                        nc.sync.dma_start(out_d[bass.ts(m, 128), :], st[:])

    nc.compile()
    return nc


def _rope_tables():
    inv_freq = 1.0 / (10000.0 ** (np.arange(0, HD, 2, dtype=np.float32) / HD))
    pos = np.arange(N, dtype=np.float32)
    freqs = pos[:, None] * inv_freq[None, :]          # [N, HD/2]
    emb = np.concatenate([freqs, freqs], axis=-1)     # [N, HD]
    cos = np.cos(emb).astype(np.float32).T.copy()     # [HD, N]
    sin = np.sin(emb).astype(np.float32).T.copy()     # [HD, N]
    sin_signed = sin.copy()
    sin_signed[0:64] *= -1.0
    return cos, sin_signed


def _make_in_maps(x, Wq, Wk, Wv, Wo):
    cos, sin_signed = _rope_tables()
    bf = ml_dtypes.bfloat16

    in_maps = []
    for c in range(N_CORES):
        b, hg = c // 4, c % 4
        cols = slice(C * hg, C * hg + C)
        xT = np.ascontiguousarray(x[b].T)                      # [D, N]
        xt = np.ascontiguousarray(
            xT.reshape(KT, 128, NB, 512).transpose(2, 1, 0, 3)
        ).astype(bf)                                           # [NB,128,KT,512]

        def wslice(W):
            wt = W[cols, :].T                                  # [D, C]
            return np.ascontiguousarray(
                wt.reshape(KT, 128, C).transpose(1, 0, 2)
            ).astype(bf)                                       # [128, KT, C]

        wo_t = Wo[:, cols].T                                   # [C, D]
        wo = np.ascontiguousarray(
            wo_t.reshape(HL, 128, D).transpose(1, 0, 2)
        ).astype(bf)                                           # [128, HL, D]

        in_maps.append({
            "xt": xt,
            "wq": wslice(Wq),
            "wk": wslice(Wk),
            "wv": wslice(Wv),
            "wo": wo,
            "cos": cos.astype(bf),
            "sin": sin_signed.astype(bf),
            "onec": np.ones((128, 1), dtype=bf),
        })
    return in_maps


def kernel(x, Wq, Wk, Wv, Wo):
    x = np.asarray(x, dtype=np.float32)
    Wq = np.asarray(Wq, dtype=np.float32)
    Wk = np.asarray(Wk, dtype=np.float32)
    Wv = np.asarray(Wv, dtype=np.float32)
    Wo = np.asarray(Wo, dtype=np.float32)

    if "nc" not in _CACHE:
        _CACHE["nc"] = _build_program()
    nc = _CACHE["nc"]

    in_maps = _make_in_maps(x, Wq, Wk, Wv, Wo)
    results = run_bass_kernel_spmd(
        nc, in_maps, core_ids=list(range(N_CORES))
    ).results

    out = np.zeros((B, N, D), dtype=np.float32)
    for c in range(N_CORES):
        out[c // 4] += np.asarray(results[c]["out"], dtype=np.float32)
    return out

